# revision 12
# baseline (speedup 1.0000x reference)
"""InfoNCE lower-bound kernel for 8 Trainium2 NeuronCores (v5).

Math (reference):
  hx = x @ W1x.T ; hy = y @ W1y.T            [N, H]
  z_ij = relu(hx[j] + hy[i] + b1) . w2       (logit WITHOUT b2)
  T1[i,j] = softplus(z_ij + b2);  T0[i] = T1[i,i]
  lse[i]  = log(N + sum_j exp(z_ij + b2))
  out     = mean(T0) - (mean(lse) - log N)

Per-core layout (64 i-rows, bf16 wide paths, fp32 PSUM):
  * H=300 -> chunks (64,64,64,64,44). i-rows in PAIRS: one [128,512] relu
    tile holds a chunk for i (partitions 0..63) and i+1 (64..127; res
    chunk at 64..107), built by one DVE/Act op from a row-duplicated hxb
    copy plus a paired per-partition bias column (hy+b1, drained from
    PSUM straight into the paired layout).
  * w2 contraction: m=2 block-diagonal matvecs on the PE, column-tiled 4
    ways; pair p accumulates into PSUM rows {32g,32g+1} of bank p//4.
    Banks double as precompute PSUM, then are DVE-zeroed; matmul
    overwrite-on-clean / accumulate-on-written semantics make any chunk
    order safe.
  * Main loop is phase-split: chunks (4,0,1) for all pairs, then (2,3),
    so the t1-derived chunks are never on the critical path.
  * Per-bank drain (overlapped): Act Exp + accum_out row sums, then
    incremental Ln of the row sums and of the 8 diagonal columns.
  * x columns are rotated by 64*core so the T1 diagonal of local row i
    sits at column i; T0 is read out of the grid itself.
"""

import math

import numpy as np
import ml_dtypes

N = 512
XD = 768
YD = 768
H = 300
NCORES = 8
ISH = N // NCORES   # 64 rows per core
KD = XD // 128      # 6 contraction tiles of 128
NPAIR = ISH // 2    # 32 pairs
CH = [64, 64, 64, 64, 44]        # h-chunk sizes
CHOFF = [0, 64, 128, 192, 256]   # h offset of each chunk
BF16 = ml_dtypes.bfloat16

_CACHE = {}
TRACE = False
LAST_RESULTS = None


def _build_module():
    import concourse.bacc as bacc
    import concourse.mybir as mybir
    from concourse.tile import TileContext

    f32 = mybir.dt.float32
    bf16 = mybir.dt.bfloat16
    AF = mybir.ActivationFunctionType
    ALU = mybir.AluOpType
    AX = mybir.AxisListType

    nc = bacc.Bacc("TRN2", target_bir_lowering=False, debug=False)

    xT = nc.dram_tensor("xT", [XD, N], bf16, kind="ExternalInput")    # x^T, cols rotated
    w1xT = nc.dram_tensor("w1xT", [XD, H], bf16, kind="ExternalInput")
    w1yT = nc.dram_tensor("w1yT", [YD, H], bf16, kind="ExternalInput")
    yT = nc.dram_tensor("yT", [YD, ISH], bf16, kind="ExternalInput")
    cf = nc.dram_tensor("cf", [128, 71], f32, kind="ExternalInput")   # b1|b2|dmask|pmask|one|N
    w2pk = nc.dram_tensor("w2pk", [128, 10], bf16, kind="ExternalInput")
    out = nc.dram_tensor("out", [1, 2], f32, kind="ExternalOutput")   # [t0_sum, lse_sum]

    with TileContext(nc) as tc:
        cpool = tc.alloc_tile_pool(name="consts", bufs=1)
        rpool = tc.alloc_tile_pool(name="rtiles", bufs=16)
        tpool = tc.alloc_tile_pool(name="tail", bufs=1)
        zpool = tc.alloc_tile_pool(name="zb", bufs=1, space="PSUM")

        # ---- persistent SBUF tiles ----
        xt_sb = cpool.tile([128, KD * N], bf16, tag="xt")
        w1x_sb = cpool.tile([128, KD * H], bf16, tag="w1x")
        w1y_sb = cpool.tile([128, KD * H], bf16, tag="w1y")
        yt_sb = cpool.tile([128, KD * ISH], bf16, tag="yt")
        cf_sb = cpool.tile([128, 71], f32, tag="cf")
        w2_sb = cpool.tile([128, 10], bf16, tag="w2")
        hyp_sb = cpool.tile([128, 5 * NPAIR], f32, tag="hyp")  # paired bias cols
        hxd_sb = cpool.tile([128, 5 * N], bf16, tag="hxd")     # row-duplicated hxb chunks
        ee_sb = cpool.tile([128, 8 * N], bf16, tag="ee")       # e^(z+b2) per bank
        acc_sb = cpool.tile([128, 8], f32, tag="acc")          # row sums per bank
        lsev_sb = cpool.tile([128, 8], f32, tag="lsev")        # ln(N + acc)
        t0ln_sb = cpool.tile([128, 64], f32, tag="t0ln")       # ln(1 + ee diag)
        zs_sb = cpool.tile([128, N], bf16, tag="zs")           # zero scratch

        b1c = cf_sb[:, 0:3]
        b2c = cf_sb[:, 3:4]
        dmaskc = cf_sb[:, 4:68]
        pmaskc = cf_sb[:, 68:69]
        onec = cf_sb[:, 69:70]
        nnc = cf_sb[:, 70:71]

        # ---- input DMAs: sync ring [yt, cf, w2, xt(2)], scalar ring [w1y, w1x]
        nc.scalar.dma_start(
            w1y_sb[:].rearrange("p (k h) -> p k h", k=KD),
            w1yT[:].rearrange("(k p) h -> p k h", p=128))
        nc.scalar.dma_start(
            w1x_sb[:].rearrange("p (k h) -> p k h", k=KD),
            w1xT[:].rearrange("(k p) h -> p k h", p=128))
        nc.sync.dma_start(
            yt_sb[:].rearrange("p (k i) -> p k i", k=KD),
            yT[:].rearrange("(k p) i -> p k i", p=128))
        nc.sync.dma_start(cf_sb[:], cf[:])
        nc.sync.dma_start(w2_sb[:], w2pk[:])
        KH = KD // 2
        nc.sync.dma_start(
            xt_sb[:, 0:KH * N].rearrange("p (k n) -> p k n", k=KH),
            xT[0:KH * 128, :].rearrange("(k p) n -> p k n", p=128))
        nc.sync.dma_start(
            xt_sb[:, KH * N:].rearrange("p (k n) -> p k n", k=KH),
            xT[KH * 128:, :].rearrange("(k p) n -> p k n", p=128))

        zb = [zpool.tile([128, N], f32, tag=f"zb{b}", name=f"zb{b}") for b in range(8)]

        # ---- early scratch + PE warmup (HAM) during the DMA window ----
        nc.vector.memset(zs_sb[:], 0.0)
        nc.vector.memset(hxd_sb[:, 4 * N:5 * N], 0.0)   # res chunk gap rows
        nc.vector.memset(hyp_sb[:], 0.0)
        for w in range(56):
            nc.tensor.matmul(
                zb[3][0:1, 0:64], lhsT=zs_sb[:, 0:1], rhs=zs_sb[:, 0:64],
                start=True, stop=True,
            )

        # ---- precompute, t-tile order (2, 0, 1) ----
        # hy tile t -> bank t (0..2); hxb tile t -> bank HXB_BANK[t] (5..7)
        HT_SZ = [128, 128, 44]
        HXB_BANK = {2: 7, 0: 5, 1: 6}
        CH_BY_T = {0: [(0, 0, 64), (1, 64, 64)], 1: [(2, 0, 64), (3, 64, 64)],
                   2: [(4, 0, 44)]}
        hyp_v = hyp_sb[:].rearrange("p (q c) -> p q c", c=5)
        first_act = True
        for t in (2, 0, 1):
            hs = HT_SZ[t]
            # hy = y @ W1y^T  (n=64)
            for k in range(KD):
                nc.tensor.matmul(
                    zb[t][0:hs, 0:ISH],
                    lhsT=w1y_sb[:, k * H + 128 * t: k * H + 128 * t + hs],
                    rhs=yt_sb[:, k * ISH:(k + 1) * ISH],
                    start=(k == 0), stop=(k == KD - 1),
                )
            # drain (+b1) straight into the paired-column layout
            for c, ro, cs in CH_BY_T[t]:
                hb = 64 if c == 4 else cs
                for half in range(2):
                    nc.vector.tensor_scalar(
                        hyp_v[half * hb:half * hb + cs, :, c],
                        zb[t][ro:ro + cs, 0:ISH][:, half::2],
                        b1c[ro:ro + cs, t:t + 1], None, ALU.add,
                    )
            nc.vector.memset(zb[t][:], 0.0)

            # hxb = W1x @ x^T  (n=512)
            bnk = HXB_BANK[t]
            for k in range(KD):
                nc.tensor.matmul(
                    zb[bnk][0:hs, :],
                    lhsT=w1x_sb[:, k * H + 128 * t: k * H + 128 * t + hs],
                    rhs=xt_sb[:, k * N:(k + 1) * N],
                    start=(k == 0), stop=(k == KD - 1),
                )
            for c, ro, cs in CH_BY_T[t]:
                hb = 64 if c == 4 else cs
                nc.scalar.activation(
                    hxd_sb[0:cs, c * N:(c + 1) * N], zb[bnk][ro:ro + cs, :],
                    AF.Identity, bias=b1c[ro:ro + cs, t:t + 1],
                )
                if first_act:
                    # preload Exp/Ln tables while Act is otherwise idle
                    tdum = tpool.tile([1, 2], f32, tag="tdum")
                    nc.scalar.activation(tdum[0:1, 0:1], onec[0:1, :], AF.Exp,
                                         bias=onec[0:1, :])
                    nc.scalar.activation(tdum[0:1, 1:2], onec[0:1, :], AF.Ln,
                                         bias=onec[0:1, :])
                    first_act = False
                nc.vector.tensor_copy(
                    hxd_sb[hb:hb + cs, c * N:(c + 1) * N],
                    hxd_sb[0:cs, c * N:(c + 1) * N],
                )
            nc.vector.memset(zb[bnk][:], 0.0)
        nc.vector.memset(zb[3][:], 0.0)
        nc.vector.memset(zb[4][:], 0.0)

        # ---- main loop: phase A chunks (4,0,1), phase B chunks (2,3) ----
        def make_tile(p, c, on_act):
            cs = 128 if c == 4 else 2 * CH[c]
            r = rpool.tile([128, N], bf16, tag="r")
            src = hxd_sb[0:cs, c * N:(c + 1) * N]
            col = hyp_sb[0:cs, 5 * p + c:5 * p + c + 1]
            if on_act:
                nc.scalar.activation(r[0:cs, :], src, AF.Relu, bias=col)
            else:
                nc.vector.tensor_scalar(r[0:cs, :], src, col, 0.0, ALU.add, ALU.max)
            return r

        def pair_mm(p, c, r):
            g = p % 4
            cs = 128 if c == 4 else 2 * CH[c]
            nc.tensor.matmul(
                zb[p // 4][32 * g:32 * g + 2, :],
                lhsT=w2_sb[0:cs, 2 * c:2 * c + 2],
                rhs=r[0:cs, :],
                start=False, stop=(c == 3),
                tile_position=(0, 32 * g),
            )

        for p in range(NPAIR):
            for c in (4, 0, 1):
                on_act = (c == 4 and p % 4 != 3) or (c == 0 and p % 8 == 1)
                pair_mm(p, c, make_tile(p, c, on_act))

        for p in range(NPAIR):
            for c in (2, 3):
                on_act = (c == 2 and p % 8 == 3)
                pair_mm(p, c, make_tile(p, c, on_act))
            if p % 4 == 3:
                b = p // 4
                nc.scalar.activation(
                    ee_sb[:, b * N:(b + 1) * N], zb[b][:], AF.Exp, bias=b2c,
                    accum_out=acc_sb[:, b:b + 1],
                )
                nc.scalar.activation(
                    lsev_sb[:, b:b + 1], acc_sb[:, b:b + 1], AF.Ln, bias=nnc)
                nc.scalar.activation(
                    t0ln_sb[:, 8 * b:8 * b + 8],
                    ee_sb[:, b * N + 8 * b:b * N + 8 * b + 8], AF.Ln, bias=onec)

        # ---- tail ----
        t0m = tpool.tile([128, 64], f32, tag="t0m")
        nc.vector.tensor_tensor(t0m[:], t0ln_sb[:], dmaskc, ALU.mult)
        combo = tpool.tile([128, 2], f32, tag="combo")
        nc.vector.tensor_reduce(combo[:, 0:1], t0m[:], axis=AX.X, op=ALU.add)
        nc.vector.tensor_reduce(combo[:, 1:2], lsev_sb[:], axis=AX.X, op=ALU.add)

        zpool.release()
        pp_tail = tc.alloc_tile_pool(name="pp_tail", bufs=1, space="PSUM")
        fps = pp_tail.tile([128, 2], f32, tag="fps")
        nc.tensor.matmul(
            fps[0:1, 0:2], lhsT=pmaskc, rhs=combo[:], start=True, stop=True,
        )
        final = tpool.tile([1, 2], f32, tag="final")
        nc.vector.tensor_copy(final[:], fps[0:1, 0:2])
        nc.sync.dma_start(out[:], final[:])

        for pl in (pp_tail, tpool, rpool, cpool):
            pl.release()

    nc.finalize()
    return nc


def _get_module():
    if "nc" not in _CACHE:
        _CACHE["nc"] = _build_module()
    return _CACHE["nc"]


def kernel(**inputs) -> np.ndarray:
    from concourse.bass_utils import run_bass_kernel_spmd

    x = np.ascontiguousarray(np.asarray(inputs["x_samples"], dtype=np.float32))
    y = np.ascontiguousarray(np.asarray(inputs["y_samples"], dtype=np.float32))
    W1 = np.asarray(inputs["W1"], dtype=np.float32)
    b1 = np.asarray(inputs["b1"], dtype=np.float32).reshape(H)
    W2 = np.asarray(inputs["W2"], dtype=np.float32)
    b2 = float(np.asarray(inputs["b2"], dtype=np.float32).reshape(1)[0])

    w1xT = np.ascontiguousarray(W1[:, :XD].T).astype(BF16)
    w1yT = np.ascontiguousarray(W1[:, XD:].T).astype(BF16)

    w2 = W2.reshape(H)
    w2pk = np.zeros((128, 10), np.float32)
    for c in range(5):
        cs = CH[c]
        hb = 64 if c == 4 else cs
        w2pk[0:cs, 2 * c] = w2[CHOFF[c]:CHOFF[c] + cs]
        w2pk[hb:hb + cs, 2 * c + 1] = w2[CHOFF[c]:CHOFF[c] + cs]
    w2pk = w2pk.astype(BF16)

    # consts: b1(3) | b2(1) | dmask(64) | pmask(1) | 1.0 | N
    cf = np.zeros((128, 71), np.float32)
    for t, hs in enumerate((128, 128, 44)):
        cf[:hs, t] = b1[128 * t:128 * t + hs]
    cf[:, 3] = b2
    for g in range(4):
        for h in range(2):
            cf[32 * g + h, 68] = 1.0                         # pmask
            for b in range(8):
                cf[32 * g + h, 4 + 8 * b + 2 * g + h] = 1.0  # dmask
    cf[:, 69] = 1.0
    cf[:, 70] = float(N)

    in_maps = []
    for c in range(NCORES):
        sl = slice(c * ISH, (c + 1) * ISH)
        xrot = np.roll(x, -c * ISH, axis=0)          # diag of row i at col i
        in_maps.append({
            "xT": np.ascontiguousarray(xrot.T).astype(BF16),
            "w1xT": w1xT,
            "w1yT": w1yT,
            "yT": np.ascontiguousarray(y[sl].T).astype(BF16),
            "cf": cf,
            "w2pk": w2pk,
        })

    nc = _get_module()
    res = run_bass_kernel_spmd(
        nc, in_maps, core_ids=list(range(NCORES)), trace=TRACE
    )
    global LAST_RESULTS
    LAST_RESULTS = res
    t0_sum = 0.0
    lse_sum = 0.0
    for r in res.results:
        o = r["out"]
        t0_sum += float(o[0, 0])
        lse_sum += float(o[0, 1])
    val = t0_sum / N - (lse_sum / N - math.log(N))
    return np.float32(val)


# revision 13
# speedup vs baseline: 1.0170x; 1.0170x over previous
"""InfoNCE lower-bound kernel for 8 Trainium2 NeuronCores (v5).

Math (reference):
  hx = x @ W1x.T ; hy = y @ W1y.T            [N, H]
  z_ij = relu(hx[j] + hy[i] + b1) . w2       (logit WITHOUT b2)
  T1[i,j] = softplus(z_ij + b2);  T0[i] = T1[i,i]
  lse[i]  = log(N + sum_j exp(z_ij + b2))
  out     = mean(T0) - (mean(lse) - log N)

Per-core layout (64 i-rows, bf16 wide paths, fp32 PSUM):
  * H=300 -> chunks (64,64,64,64,44). i-rows in PAIRS: one [128,512] relu
    tile holds a chunk for i (partitions 0..63) and i+1 (64..127; res
    chunk at 64..107), built by one DVE/Act op from a row-duplicated hxb
    copy plus a paired per-partition bias column (hy+b1, drained from
    PSUM straight into the paired layout).
  * w2 contraction: m=2 block-diagonal matvecs on the PE, column-tiled 4
    ways; pair p accumulates into PSUM rows {32g,32g+1} of bank p//4.
    Banks double as precompute PSUM, then are DVE-zeroed; matmul
    overwrite-on-clean / accumulate-on-written semantics make any chunk
    order safe.
  * Main loop is phase-split: chunks (4,0,1) for all pairs, then (2,3),
    so the t1-derived chunks are never on the critical path.
  * Per-bank drain (overlapped): Act Exp + accum_out row sums, then
    incremental Ln of the row sums and of the 8 diagonal columns.
  * x columns are rotated by 64*core so the T1 diagonal of local row i
    sits at column i; T0 is read out of the grid itself.
"""

import math

import numpy as np
import ml_dtypes

N = 512
XD = 768
YD = 768
H = 300
NCORES = 8
ISH = N // NCORES   # 64 rows per core
KD = XD // 128      # 6 contraction tiles of 128
NPAIR = ISH // 2    # 32 pairs
CH = [64, 64, 64, 64, 44]        # h-chunk sizes
CHOFF = [0, 64, 128, 192, 256]   # h offset of each chunk
BF16 = ml_dtypes.bfloat16

_CACHE = {}
TRACE = False
LAST_RESULTS = None


def _build_module():
    import concourse.bacc as bacc
    import concourse.mybir as mybir
    from concourse.tile import TileContext

    f32 = mybir.dt.float32
    bf16 = mybir.dt.bfloat16
    AF = mybir.ActivationFunctionType
    ALU = mybir.AluOpType
    AX = mybir.AxisListType

    nc = bacc.Bacc("TRN2", target_bir_lowering=False, debug=False)

    xT = nc.dram_tensor("xT", [XD, N], bf16, kind="ExternalInput")    # x^T, cols rotated
    w1xT = nc.dram_tensor("w1xT", [XD, H], bf16, kind="ExternalInput")
    w1yT = nc.dram_tensor("w1yT", [YD, H], bf16, kind="ExternalInput")
    yT = nc.dram_tensor("yT", [YD, ISH], bf16, kind="ExternalInput")
    cf = nc.dram_tensor("cf", [128, 71], f32, kind="ExternalInput")   # b1|b2|dmask|pmask|one|N
    w2pk = nc.dram_tensor("w2pk", [128, 10], bf16, kind="ExternalInput")
    out = nc.dram_tensor("out", [1, 2], f32, kind="ExternalOutput")   # [t0_sum, lse_sum]

    with TileContext(nc) as tc:
        cpool = tc.alloc_tile_pool(name="consts", bufs=1)
        rpool = tc.alloc_tile_pool(name="rtiles", bufs=16)
        tpool = tc.alloc_tile_pool(name="tail", bufs=1)
        zpool = tc.alloc_tile_pool(name="zb", bufs=1, space="PSUM")

        # ---- persistent SBUF tiles ----
        xt_sb = cpool.tile([128, KD * N], bf16, tag="xt")
        w1x_sb = cpool.tile([128, KD * H], bf16, tag="w1x")
        w1y_sb = cpool.tile([128, KD * H], bf16, tag="w1y")
        yt_sb = cpool.tile([128, KD * ISH], bf16, tag="yt")
        cf_sb = cpool.tile([128, 71], f32, tag="cf")
        w2_sb = cpool.tile([128, 10], bf16, tag="w2")
        hyp_sb = cpool.tile([128, 5 * NPAIR], f32, tag="hyp")  # paired bias cols
        hxd_sb = cpool.tile([128, 5 * N], bf16, tag="hxd")     # row-duplicated hxb chunks
        ee_sb = cpool.tile([128, 8 * N], bf16, tag="ee")       # e^(z+b2) per bank
        acc_sb = cpool.tile([128, 8], f32, tag="acc")          # row sums per bank
        lsev_sb = cpool.tile([128, 8], f32, tag="lsev")        # ln(N + acc)
        t0ln_sb = cpool.tile([128, 64], f32, tag="t0ln")       # ln(1 + ee diag)
        zs_sb = cpool.tile([128, N], bf16, tag="zs")           # zero scratch

        b1c = cf_sb[:, 0:3]
        b2c = cf_sb[:, 3:4]
        dmaskc = cf_sb[:, 4:68]
        pmaskc = cf_sb[:, 68:69]
        onec = cf_sb[:, 69:70]
        nnc = cf_sb[:, 70:71]

        # ---- input DMAs: sync ring [yt, cf, w2, xt(2)], scalar ring [w1y, w1x]
        nc.scalar.dma_start(
            w1y_sb[:].rearrange("p (k h) -> p k h", k=KD),
            w1yT[:].rearrange("(k p) h -> p k h", p=128))
        nc.scalar.dma_start(
            w1x_sb[:].rearrange("p (k h) -> p k h", k=KD),
            w1xT[:].rearrange("(k p) h -> p k h", p=128))
        nc.sync.dma_start(
            yt_sb[:].rearrange("p (k i) -> p k i", k=KD),
            yT[:].rearrange("(k p) i -> p k i", p=128))
        nc.sync.dma_start(cf_sb[:], cf[:])
        nc.sync.dma_start(w2_sb[:], w2pk[:])
        KH = KD // 2
        nc.sync.dma_start(
            xt_sb[:, 0:KH * N].rearrange("p (k n) -> p k n", k=KH),
            xT[0:KH * 128, :].rearrange("(k p) n -> p k n", p=128))
        nc.sync.dma_start(
            xt_sb[:, KH * N:].rearrange("p (k n) -> p k n", k=KH),
            xT[KH * 128:, :].rearrange("(k p) n -> p k n", p=128))

        zb = [zpool.tile([128, N], f32, tag=f"zb{b}", name=f"zb{b}") for b in range(8)]

        # ---- early scratch + PE warmup (HAM) during the DMA window ----
        nc.vector.memset(zs_sb[:], 0.0)
        nc.vector.memset(hxd_sb[:, 4 * N:5 * N], 0.0)   # res chunk gap rows
        nc.vector.memset(hyp_sb[:], 0.0)
        for w in range(56):
            nc.tensor.matmul(
                zb[3][0:1, 0:64], lhsT=zs_sb[:, 0:1], rhs=zs_sb[:, 0:64],
                start=True, stop=True,
            )

        # ---- precompute, t-tile order (2, 0, 1) ----
        # hy tile t -> bank t (0..2); hxb tile t -> bank HXB_BANK[t] (5..7)
        HT_SZ = [128, 128, 44]
        HXB_BANK = {2: 7, 0: 5, 1: 6}
        CH_BY_T = {0: [(0, 0, 64), (1, 64, 64)], 1: [(2, 0, 64), (3, 64, 64)],
                   2: [(4, 0, 44)]}
        hyp_v = hyp_sb[:].rearrange("p (q c) -> p q c", c=5)
        first_act = True
        for t in (2, 0, 1):
            hs = HT_SZ[t]
            # hy = y @ W1y^T  (n=64)
            for k in range(KD):
                nc.tensor.matmul(
                    zb[t][0:hs, 0:ISH],
                    lhsT=w1y_sb[:, k * H + 128 * t: k * H + 128 * t + hs],
                    rhs=yt_sb[:, k * ISH:(k + 1) * ISH],
                    start=(k == 0), stop=(k == KD - 1),
                )
            # drain (+b1) straight into the paired-column layout
            for c, ro, cs in CH_BY_T[t]:
                hb = 64 if c == 4 else cs
                for half in range(2):
                    nc.vector.tensor_scalar(
                        hyp_v[half * hb:half * hb + cs, :, c],
                        zb[t][ro:ro + cs, 0:ISH][:, half::2],
                        b1c[ro:ro + cs, t:t + 1], None, ALU.add,
                    )
            nc.vector.memset(zb[t][:], 0.0)

            # hxb = W1x @ x^T  (n=512)
            bnk = HXB_BANK[t]
            for k in range(KD):
                nc.tensor.matmul(
                    zb[bnk][0:hs, :],
                    lhsT=w1x_sb[:, k * H + 128 * t: k * H + 128 * t + hs],
                    rhs=xt_sb[:, k * N:(k + 1) * N],
                    start=(k == 0), stop=(k == KD - 1),
                )
            for c, ro, cs in CH_BY_T[t]:
                hb = 64 if c == 4 else cs
                nc.scalar.activation(
                    hxd_sb[0:cs, c * N:(c + 1) * N], zb[bnk][ro:ro + cs, :],
                    AF.Identity, bias=b1c[ro:ro + cs, t:t + 1],
                )
                if first_act:
                    # preload Exp/Ln tables while Act is otherwise idle
                    tdum = tpool.tile([1, 2], f32, tag="tdum")
                    nc.scalar.activation(tdum[0:1, 0:1], onec[0:1, :], AF.Exp,
                                         bias=onec[0:1, :])
                    nc.scalar.activation(tdum[0:1, 1:2], onec[0:1, :], AF.Ln,
                                         bias=onec[0:1, :])
                    first_act = False
                nc.vector.tensor_copy(
                    hxd_sb[hb:hb + cs, c * N:(c + 1) * N],
                    hxd_sb[0:cs, c * N:(c + 1) * N],
                )
            nc.vector.memset(zb[bnk][:], 0.0)
        nc.vector.memset(zb[3][:], 0.0)
        nc.vector.memset(zb[4][:], 0.0)

        # ---- main loop: phase A chunks (4,0,1), phase B chunks (2,3) ----
        def make_tile(p, c, on_act):
            cs = 128 if c == 4 else 2 * CH[c]
            r = rpool.tile([128, N], bf16, tag="r")
            src = hxd_sb[0:cs, c * N:(c + 1) * N]
            col = hyp_sb[0:cs, 5 * p + c:5 * p + c + 1]
            if on_act:
                nc.scalar.activation(r[0:cs, :], src, AF.Relu, bias=col)
            else:
                nc.vector.tensor_scalar(r[0:cs, :], src, col, 0.0, ALU.add, ALU.max)
            return r

        def pair_mm(p, c, r):
            g = p % 4
            cs = 128 if c == 4 else 2 * CH[c]
            nc.tensor.matmul(
                zb[p // 4][32 * g:32 * g + 2, :],
                lhsT=w2_sb[0:cs, 2 * c:2 * c + 2],
                rhs=r[0:cs, :],
                start=False, stop=(c == 3),
                tile_position=(0, 32 * g),
            )

        for p in range(NPAIR):
            for c in (4, 0, 1):
                on_act = (c == 4 and p % 4 != 3) or (c == 0 and p % 8 == 1)
                pair_mm(p, c, make_tile(p, c, on_act))

        pend = []
        for p in range(NPAIR):
            for c in (2, 3):
                on_act = (c == 2 and p % 2 == 1 and p < 24)
                pair_mm(p, c, make_tile(p, c, on_act))
            if pend and pend[0][0] <= p - 2:
                _, bb = pend.pop(0)
                nc.vector.tensor_reduce(
                    acc_sb[:, bb:bb + 1], ee_sb[:, bb * N:(bb + 1) * N],
                    axis=AX.X, op=ALU.add)
            if p % 4 == 3:
                b = p // 4
                nc.scalar.activation(
                    ee_sb[:, b * N:(b + 1) * N], zb[b][:], AF.Exp, bias=b2c,
                )
                nc.scalar.activation(
                    t0ln_sb[:, 8 * b:8 * b + 8],
                    ee_sb[:, b * N + 8 * b:b * N + 8 * b + 8], AF.Ln, bias=onec)
                pend.append((p, b))
        for _, bb in pend:
            nc.vector.tensor_reduce(
                acc_sb[:, bb:bb + 1], ee_sb[:, bb * N:(bb + 1) * N],
                axis=AX.X, op=ALU.add)

        # ---- tail ----
        nc.scalar.activation(lsev_sb[:], acc_sb[:], AF.Ln, bias=nnc)
        t0m = tpool.tile([128, 64], f32, tag="t0m")
        nc.vector.tensor_tensor(t0m[:], t0ln_sb[:], dmaskc, ALU.mult)
        combo = tpool.tile([128, 2], f32, tag="combo")
        nc.vector.tensor_reduce(combo[:, 0:1], t0m[:], axis=AX.X, op=ALU.add)
        nc.vector.tensor_reduce(combo[:, 1:2], lsev_sb[:], axis=AX.X, op=ALU.add)

        zpool.release()
        pp_tail = tc.alloc_tile_pool(name="pp_tail", bufs=1, space="PSUM")
        fps = pp_tail.tile([128, 2], f32, tag="fps")
        nc.tensor.matmul(
            fps[0:1, 0:2], lhsT=pmaskc, rhs=combo[:], start=True, stop=True,
        )
        final = tpool.tile([1, 2], f32, tag="final")
        nc.vector.tensor_copy(final[:], fps[0:1, 0:2])
        nc.sync.dma_start(out[:], final[:])

        for pl in (pp_tail, tpool, rpool, cpool):
            pl.release()

    nc.finalize()
    return nc


def _get_module():
    if "nc" not in _CACHE:
        _CACHE["nc"] = _build_module()
    return _CACHE["nc"]


def kernel(**inputs) -> np.ndarray:
    from concourse.bass_utils import run_bass_kernel_spmd

    x = np.ascontiguousarray(np.asarray(inputs["x_samples"], dtype=np.float32))
    y = np.ascontiguousarray(np.asarray(inputs["y_samples"], dtype=np.float32))
    W1 = np.asarray(inputs["W1"], dtype=np.float32)
    b1 = np.asarray(inputs["b1"], dtype=np.float32).reshape(H)
    W2 = np.asarray(inputs["W2"], dtype=np.float32)
    b2 = float(np.asarray(inputs["b2"], dtype=np.float32).reshape(1)[0])

    w1xT = np.ascontiguousarray(W1[:, :XD].T).astype(BF16)
    w1yT = np.ascontiguousarray(W1[:, XD:].T).astype(BF16)

    w2 = W2.reshape(H)
    w2pk = np.zeros((128, 10), np.float32)
    for c in range(5):
        cs = CH[c]
        hb = 64 if c == 4 else cs
        w2pk[0:cs, 2 * c] = w2[CHOFF[c]:CHOFF[c] + cs]
        w2pk[hb:hb + cs, 2 * c + 1] = w2[CHOFF[c]:CHOFF[c] + cs]
    w2pk = w2pk.astype(BF16)

    # consts: b1(3) | b2(1) | dmask(64) | pmask(1) | 1.0 | N
    cf = np.zeros((128, 71), np.float32)
    for t, hs in enumerate((128, 128, 44)):
        cf[:hs, t] = b1[128 * t:128 * t + hs]
    cf[:, 3] = b2
    for g in range(4):
        for h in range(2):
            cf[32 * g + h, 68] = 1.0                         # pmask
            for b in range(8):
                cf[32 * g + h, 4 + 8 * b + 2 * g + h] = 1.0  # dmask
    cf[:, 69] = 1.0
    cf[:, 70] = float(N)

    in_maps = []
    for c in range(NCORES):
        sl = slice(c * ISH, (c + 1) * ISH)
        xrot = np.roll(x, -c * ISH, axis=0)          # diag of row i at col i
        in_maps.append({
            "xT": np.ascontiguousarray(xrot.T).astype(BF16),
            "w1xT": w1xT,
            "w1yT": w1yT,
            "yT": np.ascontiguousarray(y[sl].T).astype(BF16),
            "cf": cf,
            "w2pk": w2pk,
        })

    nc = _get_module()
    res = run_bass_kernel_spmd(
        nc, in_maps, core_ids=list(range(NCORES)), trace=TRACE
    )
    global LAST_RESULTS
    LAST_RESULTS = res
    t0_sum = 0.0
    lse_sum = 0.0
    for r in res.results:
        o = r["out"]
        t0_sum += float(o[0, 0])
        lse_sum += float(o[0, 1])
    val = t0_sum / N - (lse_sum / N - math.log(N))
    return np.float32(val)
